# revision 29
# baseline (speedup 1.0000x reference)
"""Trainium2 Bass kernel for nn_CorrelationMatrix (sparse_attention).

Math: the reference builds a (b, r, h_t*w_t, h_r*w_r) correlation volume,
runs a pair of 3x3 convs over it (first over the (h_r, w_r) key grid, then
over the (h_t, w_t) query grid), a joint softmax over (r, h_r, w_r) per
query, and aggregates masked reference features.

Because the convs are linear and each acts on one side of the einsum, they
commute into the feature tensors:

    conv1 over keys    -> applied to K features:  K = conv1(fr * vr)
    conv2 over queries -> applied to Q features:  Q = conv2(ft * vt)

and the conv biases only add per-query constants, which cancel exactly in
the softmax.  The whole module collapses to flash attention:

    S = Q^T K          (4096 queries x 16384 keys, d=128)
    P = exp(S)         (no max-subtraction: |S| < ~3 by construction)
    out = V P / sum_k P,   V = fr*vr

Sharding: KEYS are sharded 8 ways (core i gets ref frame i//2, row-half
i%2 = 2048 keys); every core runs all 4096 queries against its local keys,
accumulating partial sum_k exp()*V and partial denominators.  One
ReduceScatter(add) combines the partials and lands chunk i of the queries
on core i, which normalizes and emits out[:, 512*i : 512*(i+1)].

Schedule notes (v3):
 - the mask multiplies (fr*vr, ft*vt, V masking) are folded into host-side
   prep: the device receives pre-masked frm/ftm/vm, removing ~5us of DVE
   work and two large broadcast DMAs per iteration.
 - conv taps on DVE use tensor_scalar_mul (4x DVE perf mode, 0.26ns/col)
   + tensor_add (2x mode, 0.52ns/col); the "fused" scalar_tensor_tensor
   gets no fast mode (1.04ns/col) and is only used on Pool, where every op
   costs the same ~806ns regardless.
 - conv work is split across DVE (conv1 + conv2 chunks 0,3,4,5) and the
   otherwise-idle Pool engine (conv2 chunks 1,2,6,7) so key tiles and
   early query chunks are produced fast enough to keep PE fed.
 - the first two query chunks are interleaved over key tile-pairs
   ((0,t),(1,t),...) so the PE consumes fresh conv1 tiles at half rate
   during the warmup while DVE builds them; chunks 2-7 then run
   sequentially against fully-built tiles.
 - PSUM staging evacuation runs on the Act engine (activation Copy):
   exp uses 1038ns of Act per 1280ns PE group, and the two ~590ns copies
   per chunk fit in the accumulated slack without stalling PE.
 - denominators: all 16 per-chunk M=1 ones-matmuls accumulate into a
   single PSUM row (tile_position batching gives no concurrency - PE cost
   is per-column regardless of M - so one row is simplest and makes the
   staging 1 row instead of 4).
 - input DMAs split across both HWDGE queues (SP: frm+vm, Act: w1/w2/ftm)
   so the conv1 and conv2 input paths land concurrently.
 - flash loop is software-pipelined at emission level: the next group's QK
   matmuls are emitted before this group's PV so the PE FIFO never
   head-of-line blocks on the Act exp; exp covers two key tiles (two PSUM
   banks, 1024 wide) per instruction.
"""

import os
import numpy as np
import ml_dtypes

import concourse.bass as bass
import concourse.tile as tile
from concourse import bacc, mybir
from concourse.bass_utils import run_bass_kernel_spmd

BF16 = mybir.dt.bfloat16
F32 = mybir.dt.float32
AF = mybir.ActivationFunctionType
ALU = mybir.AluOpType

C = 128          # channels (= contraction dim = SBUF partitions)
R = 4            # reference frames
H = W = 64       # spatial grid
HW = H * W       # 4096
NK = R * HW      # 16384 keys total
NCORES = 8
NQ = 512              # queries per output chunk (and per core's RS slice)
PW = 66               # padded width for 3x3 conv (1 zero col each side)
KROWS = 32            # key rows per core
KPAD = (KROWS + 2) * PW   # 2244: padded local fr window (1 halo row each side)
NKL = KROWS * W       # 2048 local keys
KT = NKL // 128       # 16 local key tiles
QROWS = 8             # query rows per chunk
FTPAD = PW * PW       # 4356: full padded ft
NCHUNK = 8            # query chunks (one per core in the RS)
SROWS = C + 1         # stage rows per chunk: 128 out + 1 denominator row


def build_nc(loop_n: int = 1):
    nc = bacc.Bacc(None, target_bir_lowering=False, debug=False)

    frm_d = nc.declare_dram_parameter("frm", [C, KPAD], BF16, isOutput=False)
    ftm_d = nc.declare_dram_parameter("ftm", [C, FTPAD], BF16, isOutput=False)
    vm_d = nc.declare_dram_parameter("vm", [128, NKL], BF16, isOutput=False)
    w1_d = nc.declare_dram_parameter("w1", [9], F32, isOutput=False)
    w2_d = nc.declare_dram_parameter("w2", [9], F32, isOutput=False)
    out_d = nc.declare_dram_parameter("out", [C, NQ], F32, isOutput=True)

    with tile.TileContext(nc) as tc:
        with (
            tc.tile_pool(name="big", bufs=1) as big,
            tc.tile_pool(name="db", bufs=2) as db,
            tc.tile_pool(name="pp", bufs=4) as pp,
            tc.tile_pool(name="stg", bufs=3) as stg,
            tc.tile_pool(name="ps_s", bufs=2, space="PSUM") as ps_s,
            tc.tile_pool(name="ps_o", bufs=3, space="PSUM") as ps_o,
            tc.tile_pool(name="ps_l", bufs=1, space="PSUM") as ps_l,
            tc.tile_pool(name="dram", bufs=1, space="DRAM") as dram,
        ):
          import contextlib
          # constants, allocated once outside the timing loop
          ones_col = big.tile([128, 1], BF16)
          nc.vector.memset(ones_col[:, :], 1.0)
          stage_all = dram.tile([NCHUNK * SROWS, NQ], BF16)
          loop_cm = tc.For_i(0, loop_n, 1) if loop_n > 1 else contextlib.nullcontext()
          with loop_cm:
              # input loads: HWDGE descriptor-gen and the DMA engine are both
              # effectively serial shared resources, AND dependency tracking
              # for DMA-written tiles is whole-tile, so each need-ordered
              # piece gets its OWN SBUF tile (with a 2-row overlap re-read so
              # every conv piece reads exactly one input tile).
              w1_sb = db.tile([128, 9], F32)
              w2_sb = db.tile([128, 9], F32)
              FRA = 14     # frm rows 0-13 -> frma; rows 12-33 -> frmb
              FTA = 10     # ftm rows 0-9 -> ftma; rows 8-65 -> ftmb
              frma = db.tile([C, FRA * PW], BF16)
              frmb = db.tile([C, (KROWS + 2 - FRA + 2) * PW], BF16)
              ftma = db.tile([C, FTA * PW], BF16)
              ftmb = db.tile([C, (PW - FTA + 2) * PW], BF16)
              vm = db.tile([128, NKL], BF16)
              nc.sync.dma_start(
                  out=frma[:, :], in_=frm_d[:, 0 : FRA * PW])
              nc.scalar.dma_start(
                  out=w1_sb[:, :],
                  in_=bass.AP(tensor=w1_d, offset=0, ap=[[0, 128], [1, 9]]),
              )
              nc.scalar.dma_start(
                  out=w2_sb[:, :],
                  in_=bass.AP(tensor=w2_d, offset=0, ap=[[0, 128], [1, 9]]),
              )
              nc.scalar.dma_start(
                  out=ftma[:, :], in_=ftm_d[:, 0 : FTA * PW])
              nc.sync.dma_start(
                  out=frmb[:, :], in_=frm_d[:, (FRA - 2) * PW : KPAD])
              nc.scalar.dma_start(
                  out=ftmb[:, :], in_=ftm_d[:, (FTA - 2) * PW : FTPAD])
              nc.scalar.dma_start(out=vm[:, :], in_=vm_d[:, :])

              frma3 = frma[:, :].rearrange("p (r c) -> p r c", c=PW)
              frmb3 = frmb[:, :].rearrange("p (r c) -> p r c", c=PW)
              ftma3 = ftma[:, :].rearrange("p (r c) -> p r c", c=PW)
              ftmb3 = ftmb[:, :].rearrange("p (r c) -> p r c", c=PW)

              # conv outputs also get one tile per piece / per query chunk so
              # every consumer's dependency is exact.
              C1_PIECES = [(0, 4), (4, 8), (12, 8), (20, 8), (28, 4)]
              fr1p = [
                  db.tile([C, n * W], BF16, name=f"fr1p{i}")
                  for i, (_, n) in enumerate(C1_PIECES)
              ]
              fr1pv = [
                  t[:, :].rearrange("p (j x) -> p j x", x=W) for t in fr1p
              ]
              # key tile t (rows 2t, 2t+1) -> (piece index, column offset)
              tile_loc = {}
              for pi, (j0, n) in enumerate(C1_PIECES):
                  for t in range(j0 // 2, (j0 + n) // 2):
                      tile_loc[t] = (pi, (2 * t - j0) * W)
              ft2c = [
                  db.tile([C, NQ], BF16, name=f"ft2c{i}")
                  for i in range(NCHUNK)
              ]
              ft2cv = [
                  t[:, :].rearrange("p (j x) -> p j x", x=W) for t in ft2c
              ]
              # ONE shared tmp for both convs on DVE: the WAR chain through it
              # pins the DVE conv stream to emission order (the Tile scheduler
              # otherwise interleaves the independent streams, delaying the
              # completion of every piece)
              tmp_k = db.tile([C, 8 * W], BF16)
              tmp_kv = tmp_k[:, :].rearrange("p (j x) -> p j x", x=W)
              tmp_p = db.tile([C, 8 * W], BF16)
              tmp_pv = tmp_p[:, :].rearrange("p (j x) -> p j x", x=W)

              def conv_piece(dstv, d0, src3, s0, w_sb, j0, nrows, eng):
                  # conv output rows [j0, j0+nrows) into dstv rows j0-d0...;
                  # src3 holds input rows starting at absolute row s0.
                  # On DVE use mul(4x mode) + add(2x mode) pairs, on Pool the
                  # fused form (flat cost there).
                  dst = dstv[:, j0 - d0 : j0 - d0 + nrows, :]
                  on_pool = eng is nc.gpsimd
                  taps = [1, 2, 3, 4, 5, 6, 7, 8, 0]
                  for ti, tap in enumerate(taps):
                      dy, dx = divmod(tap, 3)
                      src = src3[
                          :, j0 + dy - s0 : j0 + dy - s0 + nrows, dx : dx + W
                      ]
                      wap = w_sb[:, tap : tap + 1]
                      if on_pool:
                          # GPSIMD runs only TensorTensor/Memset on real hw:
                          # multiply by a free-broadcast view of the weight
                          wb = wap.rearrange("p (a b) -> p a b", a=1)
                          wb = wb.broadcast_to([128, nrows, W])
                          if ti == 0:
                              eng.tensor_mul(dst, src, wb)
                          else:
                              tv = tmp_pv[:, 0:nrows, :]
                              eng.tensor_mul(tv, src, wb)
                              eng.tensor_add(dst, dst, tv)
                      elif ti == 0:
                          eng.tensor_scalar_mul(dst, src, wap)
                      elif tap == 0:
                          # last tap fused (dst += src*w): slower per-op but
                          # keeps every DVE op on the piece's tmp/dst chain so
                          # the scheduler cannot hoist it ahead of its inputs
                          eng.scalar_tensor_tensor(
                              dst, src, wap, dst, ALU.mult, ALU.add
                          )
                      else:
                          tv = tmp_kv[:, 0:nrows, :]
                          eng.tensor_scalar_mul(tv, src, wap)
                          eng.tensor_add(dst, dst, tv)

              def conv1_piece(pi, eng):
                  j0, n = C1_PIECES[pi]
                  src3, s0 = (frma3, 0) if j0 + n + 1 < FRA else (frmb3, FRA - 2)
                  conv_piece(fr1pv[pi], j0, src3, s0, w1_sb, j0, n, eng)

              def conv2_rows(j0, nrows, eng):
                  qc = j0 // QROWS
                  src3, s0 = (ftma3, 0) if j0 + nrows + 1 < FTA else (ftmb3, FTA - 2)
                  conv_piece(
                      ft2cv[qc], qc * QROWS, src3, s0, w2_sb, j0, nrows, eng
                  )

              # ---- conv prologue ----
              # Pool stream (independent FIFO): the three LAST query chunks -
              # TensorTensor-based taps are ~4x slower than DVE's, but these
              # aren't consumed until ~60-80us in, and Pool runs concurrently.
              for pc in (5, 6, 7):
                  conv2_rows(pc * QROWS, QROWS, nc.gpsimd)
              # DVE prologue: tiles 0,1 -> chunk 0.  The rest of the DVE conv
              # stream is fed at group boundaries so emission order tracks
              # consumption order; chunks 3/4 are deferred until after the
              # first staging copies so the ps_o/ps_l buffers recycle in time.
              conv1_piece(0, nc.vector)
              conv2_rows(0, QROWS, nc.vector)

              dve_feed = {
                  0: ("c2", 1 * QROWS, QROWS),
                  1: ("c1", 1, 0),
                  2: ("c1", 2, 0),
                  3: ("c1", 3, 0),
                  4: ("c1", 4, 0),
                  5: ("c2", 2 * QROWS, QROWS),
                  12: ("c2", 3 * QROWS, QROWS),
                  20: ("c2", 4 * QROWS, QROWS),
              }

              # ---- flash loop, software-pipelined emission ----
              # chunks 0 and 1 interleaved over tile-pairs (PE consumes each
              # fresh conv1 tile-pair twice, halving the demand rate while
              # DVE builds the rest), then chunks 2-7 sequentially.
              groups = [(qc, tt) for tt in range(0, KT, 2) for qc in (0, 1)]
              groups += [(qc, tt) for qc in range(2, NCHUNK) for tt in range(0, KT, 2)]
              NGRP = len(groups)   # 64
              s2_of = {}
              out_ps_of = {}

              def emit_qk(gi):
                  qc, tt = groups[gi]
                  s2 = ps_s.tile([128, 2 * NQ], F32, tag="s2", name="s2")
                  s2_of[gi] = s2
                  for h in range(2):
                      t = tt + h
                      pi, off = tile_loc[t]
                      nc.tensor.matmul(
                          s2[:, h * NQ : (h + 1) * NQ],
                          lhsT=fr1p[pi][:, off : off + 128],
                          rhs=ft2c[qc][:, :],
                          start=True,
                          stop=True,
                      )

              l1_all = ps_l.tile([128, NQ], F32, tag="l1_all", name="l1_all")
              emit_qk(0)
              for gi in range(NGRP):
                  qc, tt = groups[gi]
                  if tt == 0:
                      out_ps_of[qc] = ps_o.tile([C, NQ], F32, tag="out_ps", name="out_ps")
                  out_ps = out_ps_of[qc]
                  lrow = 32 * (qc % 4)

                  # prefetch-emit the next group's QK so the PE FIFO never
                  # head-of-line blocks on this group's exp.  Ahead of it,
                  # drain one piece of the DVE conv feed.
                  if gi + 1 < NGRP:
                      if gi in dve_feed:
                          kind, j0, nrows = dve_feed[gi]
                          if kind == "c1":
                              conv1_piece(j0, nc.vector)
                          else:
                              conv2_rows(j0, nrows, nc.vector)
                      emit_qk(gi + 1)

                  s2 = s2_of.pop(gi)
                  p2 = pp.tile([128, 2 * NQ], BF16, tag="p2", name="p2")
                  nc.scalar.activation(p2[:, :], s2[:, :], AF.Exp)
                  for h in range(2):
                      t = tt + h
                      nc.tensor.matmul(
                          out_ps[:, :],
                          lhsT=vm[:, t * C : (t + 1) * C],
                          rhs=p2[:, h * NQ : (h + 1) * NQ],
                          start=(t == 0),
                          stop=(t == KT - 1),
                      )
                  # denominator: accumulate every tile's column-sums into one
                  # PSUM row (M=1 matmul costs the same as any other per
                  # column).  All 8 chunks share one PSUM bank, rotating over
                  # partitions {0,32,64,96} (chunks c and c+4 reuse a row ~40us
                  # apart, after the earlier chunk's staging copy).
                  for h in range(2):
                      t = tt + h
                      nc.tensor.matmul(
                          l1_all[lrow : lrow + 1, :],
                          lhsT=ones_col[:, :],
                          rhs=p2[:, h * NQ : (h + 1) * NQ],
                          start=(t == 0),
                          stop=(t == KT - 1),
                          tile_position=(0, lrow),
                      )

                  if tt == KT - 2:
                      # end of chunk: stage partials (bf16) for the collective;
                      # PSUM evacuation on DVE (GPSIMD cannot read PSUM on
                      # real hardware; DVE has slack once the conv feed winds
                      # down, and ps_o/ps_l double-buffering covers the lag)
                      o_sb = stg.tile([C, NQ], BF16, tag="o_sb", name="o_sb")
                      nc.vector.tensor_copy(o_sb[:, :], out_ps[:, :])
                      l1_sb = stg.tile([128, NQ], BF16, tag="l1_sb", name="l1_sb")
                      nc.vector.tensor_copy(
                          l1_sb[lrow : lrow + 1, :],
                          l1_all[lrow : lrow + 1, :],
                      )
                      base = qc * SROWS
                      nc.sync.dma_start(
                          out=stage_all[base : base + C, :], in_=o_sb[:, :]
                      )
                      nc.sync.dma_start(
                          out=stage_all[base + C : base + C + 1, :],
                          in_=l1_sb[lrow : lrow + 1, :],
                      )

          # ---- combine partials across cores; chunk i lands on core i ----
          red = dram.tile([SROWS, NQ], BF16)
          nc.gpsimd.collective_compute(
              "ReduceScatter",
              ALU.add,
              replica_groups=[list(range(NCORES))],
              ins=[stage_all[:, :]],
              outs=[red[:, :]],
          )

          # ---- normalize my chunk ----
          # numerator and (partition-broadcast) denominator row loaded
          # concurrently on the two HWDGE queues; the broadcast happens in
          # the DMA itself so no PE/matmul step is needed.
          osb = big.tile([C, NQ], BF16)
          nc.sync.dma_start(out=osb[:, :], in_=red[0:C, :])
          l1b = big.tile([C, NQ], BF16)
          nc.scalar.dma_start(
              out=l1b[:, :],
              in_=red[C : C + 1, :].partition_broadcast(128),
          )
          linv = big.tile([C, NQ], F32)
          nc.vector.reciprocal(linv[:, :], l1b[:, :])
          outf = big.tile([C, NQ], F32)
          nc.vector.tensor_mul(outf[:, :], osb[:, :], linv[:, :])
          nc.sync.dma_start(out=out_d[:, :], in_=outf[:, :])

    nc.finalize()
    return nc


def prep_inputs(feats_t, feats_ref, v_t, v_ref, conv1_w, conv1_b, conv2_w,
                conv2_b):
    bf = ml_dtypes.bfloat16
    ft = np.asarray(feats_t, np.float32)[0]            # (128, 64, 64)
    fr = np.asarray(feats_ref, np.float32)[0]          # (128, 4, 64, 64)
    vt = np.asarray(v_t, np.float32)[0, 0][::4, ::4]   # (64, 64)
    vr = np.asarray(v_ref, np.float32)[0, 0][:, ::4, ::4]  # (4, 64, 64)
    w1 = np.asarray(conv1_w, np.float32).reshape(9)
    w2 = np.asarray(conv2_w, np.float32).reshape(9)

    ftm_full = (ft * vt).astype(bf)                    # (128, 64, 64) masked
    frm_full = (fr * vr[None]).astype(bf)              # (128, 4, 64, 64) masked

    # full padded masked ft (shared by all cores)
    ftp = np.zeros((C, PW, PW), bf)
    ftp[:, 1:65, 1:65] = ftm_full
    ftp = ftp.reshape(C, FTPAD)

    in_maps = []
    for i in range(NCORES):
        r = i // 2
        y0 = (i % 2) * KROWS
        # padded local masked fr window: rows y0-1 .. y0+KROWS, 66 wide
        frp = np.zeros((C, KROWS + 2, PW), bf)
        ylo = max(0, y0 - 1)
        yhi = min(H, y0 + KROWS + 1)
        frp[:, (ylo - (y0 - 1)) : (yhi - (y0 - 1)), 1:65] = frm_full[:, r, ylo:yhi, :]

        # local masked V in (k%128, t, c) layout
        frl = frm_full[:, r, y0 : y0 + KROWS, :].reshape(C, NKL).astype(np.float32)
        vdev = np.ascontiguousarray(
            frl.reshape(C, KT, 128).transpose(2, 1, 0)
        ).reshape(128, NKL).astype(bf)

        in_maps.append({
            "frm": frp.reshape(C, KPAD),
            "ftm": ftp,
            "vm": vdev,
            "w1": w1,
            "w2": w2,
        })
    return in_maps


_CACHE = {}


def _get_runner():
    """Build the SPMD executable once; repeat kernel() calls reuse it."""
    if "fn" in _CACHE:
        return _CACHE["fn"]
    import jax
    from jax.sharding import Mesh, PartitionSpec
    from jax.experimental.shard_map import shard_map
    from concourse.bass2jax import (
        install_neuronx_cc_hook, _bass_exec_p, partition_id_tensor,
    )

    install_neuronx_cc_hook()
    nc = build_nc()
    pname = nc.partition_id_tensor.name if nc.partition_id_tensor else None
    in_names, out_names, out_avals, zero_outs = [], [], [], []
    for alloc in nc.m.functions[0].allocations:
        if not isinstance(alloc, mybir.MemoryLocationSet):
            continue
        name = alloc.memorylocations[0].name
        if alloc.kind == "ExternalInput":
            if name != pname:
                in_names.append(name)
        elif alloc.kind == "ExternalOutput":
            out_names.append(name)
            shape = tuple(alloc.tensor_shape)
            dtype = mybir.dt.np(alloc.dtype)
            out_avals.append(jax.core.ShapedArray(shape, dtype))
            zero_outs.append(np.zeros(shape, dtype))
    n_params = len(in_names)
    all_names = in_names + out_names + ([pname] if pname else [])

    def _body(*args):
        operands = list(args)
        if pname is not None:
            operands.append(partition_id_tensor())
        return tuple(_bass_exec_p.bind(
            *operands,
            out_avals=tuple(out_avals),
            in_names=tuple(all_names),
            out_names=tuple(out_names),
            lowering_input_output_aliases=(),
            sim_require_finite=True,
            sim_require_nnan=True,
            nc=nc,
        ))

    devices = jax.devices()[:NCORES]
    mesh = Mesh(np.asarray(devices), ("core",))
    n_outs = len(out_avals)
    fn = jax.jit(
        shard_map(
            _body, mesh=mesh,
            in_specs=(PartitionSpec("core"),) * (n_params + n_outs),
            out_specs=(PartitionSpec("core"),) * n_outs,
            check_rep=False,
        ),
        donate_argnums=tuple(range(n_params, n_params + n_outs)),
        keep_unused=True,
    )

    def run(in_maps):
        concat = [
            np.concatenate([np.asarray(m[n]) for m in in_maps], axis=0)
            for n in in_names
        ]
        zeros = [
            np.zeros((NCORES * z.shape[0], *z.shape[1:]), z.dtype)
            for z in zero_outs
        ]
        arrs = fn(*concat, *zeros)
        return [
            {
                name: np.asarray(arrs[i]).reshape(
                    NCORES, *out_avals[i].shape
                )[c]
                for i, name in enumerate(out_names)
            }
            for c in range(NCORES)
        ]

    _CACHE["fn"] = run
    return run


def kernel(**inputs) -> np.ndarray:
    run = _get_runner()
    in_maps = prep_inputs(**inputs)
    results = run(in_maps)
    out = np.empty((C, H * W), np.float32)
    for i in range(NCORES):
        out[:, i * NQ : (i + 1) * NQ] = results[i]["out"]
    return out.reshape(1, C, H, W)


# revision 35
# speedup vs baseline: 2.1432x; 2.1432x over previous
"""Trainium2 Bass kernel for nn_CorrelationMatrix (sparse_attention).

Math: the reference builds a (b, r, h_t*w_t, h_r*w_r) correlation volume,
runs a pair of 3x3 convs over it (first over the (h_r, w_r) key grid, then
over the (h_t, w_t) query grid), a joint softmax over (r, h_r, w_r) per
query, and aggregates masked reference features.

Because the convs are linear and each acts on one side of the einsum, they
commute into the feature tensors:

    conv1 over keys    -> applied to K features:  K = conv1(fr * vr)
    conv2 over queries -> applied to Q features:  Q = conv2(ft * vt)

and the conv biases only add per-query constants, which cancel exactly in
the softmax.  The whole module collapses to flash attention:

    S = Q^T K          (4096 queries x 16384 keys, d=128)
    P = exp(S)         (no max-subtraction: |S| < ~3 by construction)
    out = V P / sum_k P,   V = fr*vr

Sharding: KEYS are sharded 8 ways (core i gets ref frame i//2, row-half
i%2 = 2048 keys); every core runs all 4096 queries against its local keys,
accumulating partial sum_k exp()*V and partial denominators.  One
ReduceScatter(add) combines the partials and lands chunk i of the queries
on core i, which normalizes and emits out[:, 512*i : 512*(i+1)].

Schedule notes (v3):
 - the mask multiplies (fr*vr, ft*vt, V masking) are folded into host-side
   prep: the device receives pre-masked frm/ftm/vm, removing ~5us of DVE
   work and two large broadcast DMAs per iteration.
 - conv taps on DVE use tensor_scalar_mul (4x DVE perf mode, 0.26ns/col)
   + tensor_add (2x mode, 0.52ns/col); the "fused" scalar_tensor_tensor
   gets no fast mode (1.04ns/col) and is only used on Pool, where every op
   costs the same ~806ns regardless.
 - conv work is split across DVE (conv1 + conv2 chunks 0,3,4,5) and the
   otherwise-idle Pool engine (conv2 chunks 1,2,6,7) so key tiles and
   early query chunks are produced fast enough to keep PE fed.
 - the first two query chunks are interleaved over key tile-pairs
   ((0,t),(1,t),...) so the PE consumes fresh conv1 tiles at half rate
   during the warmup while DVE builds them; chunks 2-7 then run
   sequentially against fully-built tiles.
 - PSUM staging evacuation runs on the Act engine (activation Copy):
   exp uses 1038ns of Act per 1280ns PE group, and the two ~590ns copies
   per chunk fit in the accumulated slack without stalling PE.
 - denominators: all 16 per-chunk M=1 ones-matmuls accumulate into a
   single PSUM row (tile_position batching gives no concurrency - PE cost
   is per-column regardless of M - so one row is simplest and makes the
   staging 1 row instead of 4).
 - input DMAs split across both HWDGE queues (SP: frm+vm, Act: w1/w2/ftm)
   so the conv1 and conv2 input paths land concurrently.
 - flash loop is software-pipelined at emission level: the next group's QK
   matmuls are emitted before this group's PV so the PE FIFO never
   head-of-line blocks on the Act exp; exp covers two key tiles (two PSUM
   banks, 1024 wide) per instruction.
"""

import os
import numpy as np
import ml_dtypes

import concourse.bass as bass
import concourse.tile as tile
from concourse import bacc, mybir
from concourse.bass_utils import run_bass_kernel_spmd

BF16 = mybir.dt.bfloat16
F32 = mybir.dt.float32
AF = mybir.ActivationFunctionType
ALU = mybir.AluOpType

C = 128          # channels (= contraction dim = SBUF partitions)
R = 4            # reference frames
H = W = 64       # spatial grid
HW = H * W       # 4096
NK = R * HW      # 16384 keys total
NCORES = 8
NQ = 512              # queries per output chunk (and per core's RS slice)
PW = 66               # padded width for 3x3 conv (1 zero col each side)
KROWS = 32            # key rows per core
KPAD = (KROWS + 2) * PW   # 2244: padded local fr window (1 halo row each side)
NKL = KROWS * W       # 2048 local keys
KT = NKL // 128       # 16 local key tiles
QROWS = 8             # query rows per chunk
FTPAD = PW * PW       # 4356: full padded ft
NCHUNK = 8            # query chunks (one per core in the RS)
SROWS = C + 1         # stage rows per chunk: 128 out + 1 denominator row


def build_nc(loop_n: int = 1):
    nc = bacc.Bacc(None, target_bir_lowering=False, debug=False)

    frm_d = nc.declare_dram_parameter("frm", [C, KPAD], BF16, isOutput=False)
    ftm_d = nc.declare_dram_parameter("ftm", [C, FTPAD], BF16, isOutput=False)
    vm_d = nc.declare_dram_parameter("vm", [128, NKL], BF16, isOutput=False)
    w1_d = nc.declare_dram_parameter("w1", [9], F32, isOutput=False)
    w2_d = nc.declare_dram_parameter("w2", [9], F32, isOutput=False)
    out_d = nc.declare_dram_parameter("out", [C, NQ], F32, isOutput=True)

    with tile.TileContext(nc) as tc:
        with (
            tc.tile_pool(name="big", bufs=1) as big,
            tc.tile_pool(name="db", bufs=2) as db,
            tc.tile_pool(name="pp", bufs=4) as pp,
            tc.tile_pool(name="stg", bufs=3) as stg,
            tc.tile_pool(name="ps_s", bufs=2, space="PSUM") as ps_s,
            tc.tile_pool(name="ps_o", bufs=3, space="PSUM") as ps_o,
            tc.tile_pool(name="ps_l", bufs=1, space="PSUM") as ps_l,
            tc.tile_pool(name="dram", bufs=1, space="DRAM") as dram,
        ):
          import contextlib
          # constants, allocated once outside the timing loop
          ones_col = big.tile([128, 1], BF16)
          nc.vector.memset(ones_col[:, :], 1.0)
          stage_all = dram.tile([NCHUNK * SROWS, NQ], BF16)
          loop_cm = tc.For_i(0, loop_n, 1) if loop_n > 1 else contextlib.nullcontext()
          with loop_cm:
              # input loads: HWDGE descriptor-gen and the DMA engine are both
              # effectively serial shared resources, AND dependency tracking
              # for DMA-written tiles is whole-tile, so each need-ordered
              # piece gets its OWN SBUF tile (with a 2-row overlap re-read so
              # every conv piece reads exactly one input tile).
              w1_sb = db.tile([128, 9], F32)
              w2_sb = db.tile([128, 9], F32)
              FRA = 14     # frm rows 0-13 -> frma; rows 12-33 -> frmb
              FTA = 10     # ftm rows 0-9 -> ftma; rows 8-65 -> ftmb
              frma = db.tile([C, FRA * PW], BF16)
              frmb = db.tile([C, (KROWS + 2 - FRA + 2) * PW], BF16)
              ftma = db.tile([C, FTA * PW], BF16)
              ftmb = db.tile([C, (PW - FTA + 2) * PW], BF16)
              vm = db.tile([128, NKL], BF16)
              nc.sync.dma_start(
                  out=frma[:, :], in_=frm_d[:, 0 : FRA * PW])
              nc.scalar.dma_start(
                  out=w1_sb[:, :],
                  in_=bass.AP(tensor=w1_d, offset=0, ap=[[0, 128], [1, 9]]),
              )
              nc.scalar.dma_start(
                  out=w2_sb[:, :],
                  in_=bass.AP(tensor=w2_d, offset=0, ap=[[0, 128], [1, 9]]),
              )
              nc.scalar.dma_start(
                  out=ftma[:, :], in_=ftm_d[:, 0 : FTA * PW])
              nc.sync.dma_start(
                  out=frmb[:, :], in_=frm_d[:, (FRA - 2) * PW : KPAD])
              nc.scalar.dma_start(
                  out=ftmb[:, :], in_=ftm_d[:, (FTA - 2) * PW : FTPAD])
              nc.scalar.dma_start(out=vm[:, :], in_=vm_d[:, :])

              frma3 = frma[:, :].rearrange("p (r c) -> p r c", c=PW)
              frmb3 = frmb[:, :].rearrange("p (r c) -> p r c", c=PW)
              ftma3 = ftma[:, :].rearrange("p (r c) -> p r c", c=PW)
              ftmb3 = ftmb[:, :].rearrange("p (r c) -> p r c", c=PW)

              # conv outputs also get one tile per piece / per query chunk so
              # every consumer's dependency is exact.
              C1_PIECES = [(0, 4), (4, 8), (12, 8), (20, 8), (28, 4)]
              fr1p = [
                  db.tile([C, n * W], BF16, name=f"fr1p{i}")
                  for i, (_, n) in enumerate(C1_PIECES)
              ]
              fr1pv = [
                  t[:, :].rearrange("p (j x) -> p j x", x=W) for t in fr1p
              ]
              # key tile t (rows 2t, 2t+1) -> (piece index, column offset)
              tile_loc = {}
              for pi, (j0, n) in enumerate(C1_PIECES):
                  for t in range(j0 // 2, (j0 + n) // 2):
                      tile_loc[t] = (pi, (2 * t - j0) * W)
              ft2c = [
                  db.tile([C, NQ], BF16, name=f"ft2c{i}")
                  for i in range(NCHUNK)
              ]
              ft2cv = [
                  t[:, :].rearrange("p (j x) -> p j x", x=W) for t in ft2c
              ]
              # ONE shared tmp for both convs on DVE: the WAR chain through it
              # pins the DVE conv stream to emission order (the Tile scheduler
              # otherwise interleaves the independent streams, delaying the
              # completion of every piece)
              tmp_k = db.tile([C, 8 * W], BF16)
              tmp_kv = tmp_k[:, :].rearrange("p (j x) -> p j x", x=W)
              tmp_p = db.tile([C, 8 * W], BF16)
              tmp_pv = tmp_p[:, :].rearrange("p (j x) -> p j x", x=W)

              def conv_piece(dstv, d0, src3, s0, w_sb, j0, nrows, eng):
                  # conv output rows [j0, j0+nrows) into dstv rows j0-d0...;
                  # src3 holds input rows starting at absolute row s0.
                  # On DVE use mul(4x mode) + add(2x mode) pairs, on Pool the
                  # fused form (flat cost there).
                  dst = dstv[:, j0 - d0 : j0 - d0 + nrows, :]
                  on_pool = eng is nc.gpsimd
                  taps = [1, 2, 3, 4, 5, 6, 7, 8, 0]
                  if int(os.environ.get("TAP0_FIRST", "1")):
                      taps = [0, 1, 2, 3, 4, 5, 6, 7, 8]
                  for ti, tap in enumerate(taps):
                      dy, dx = divmod(tap, 3)
                      src = src3[
                          :, j0 + dy - s0 : j0 + dy - s0 + nrows, dx : dx + W
                      ]
                      wap = w_sb[:, tap : tap + 1]
                      if on_pool:
                          # GPSIMD runs only TensorTensor/Memset on real hw:
                          # multiply by a free-broadcast view of the weight
                          wb = wap.rearrange("p (a b) -> p a b", a=1)
                          wb = wb.broadcast_to([128, nrows, W])
                          if ti == 0:
                              eng.tensor_mul(dst, src, wb)
                          else:
                              tv = tmp_pv[:, 0:nrows, :]
                              eng.tensor_mul(tv, src, wb)
                              eng.tensor_add(dst, dst, tv)
                      elif ti == 0 and tap != 0:
                          eng.tensor_scalar_mul(dst, src, wap)
                      elif ti == 0 and tap == 0:
                          eng.tensor_scalar_mul(dst, src, wap)
                      elif tap == 0:
                          # last tap fused (dst += src*w): slower per-op but
                          # keeps every DVE op on the piece's tmp/dst chain so
                          # the scheduler cannot hoist it ahead of its inputs
                          eng.scalar_tensor_tensor(
                              dst, src, wap, dst, ALU.mult, ALU.add
                          )
                      else:
                          tv = tmp_kv[:, 0:nrows, :]
                          eng.tensor_scalar_mul(tv, src, wap)
                          eng.tensor_add(dst, dst, tv)

              def conv1_piece(pi, eng):
                  j0, n = C1_PIECES[pi]
                  src3, s0 = (frma3, 0) if j0 + n + 1 < FRA else (frmb3, FRA - 2)
                  conv_piece(fr1pv[pi], j0, src3, s0, w1_sb, j0, n, eng)

              def conv2_rows(j0, nrows, eng):
                  qc = j0 // QROWS
                  src3, s0 = (ftma3, 0) if j0 + nrows + 1 < FTA else (ftmb3, FTA - 2)
                  conv_piece(
                      ft2cv[qc], qc * QROWS, src3, s0, w2_sb, j0, nrows, eng
                  )

              # ---- conv prologue ----
              # Pool stream (independent FIFO): the three LAST query chunks -
              # TensorTensor-based taps are ~4x slower than DVE's, but these
              # aren't consumed until ~60-80us in, and Pool runs concurrently.
              POOL_CONV = int(os.environ.get("POOL_CONV", "0"))
              if POOL_CONV:
                  for pc in (5, 6, 7):
                      conv2_rows(pc * QROWS, QROWS, nc.gpsimd)
              # DVE prologue: tiles 0,1 -> chunk 0.  The rest of the DVE conv
              # stream is fed at group boundaries so emission order tracks
              # consumption order; chunks 3/4 are deferred until after the
              # first staging copies so the ps_o/ps_l buffers recycle in time.
              conv1_piece(0, nc.vector)
              conv2_rows(0, QROWS, nc.vector)

              dve_feed = {
                  0: ("c2", 1 * QROWS, QROWS),
                  1: ("c1", 1, 0),
                  2: ("c1", 2, 0),
                  3: ("c1", 3, 0),
                  4: ("c1", 4, 0),
                  5: ("c2", 2 * QROWS, QROWS),
                  12: ("c2", 3 * QROWS, QROWS),
                  20: ("c2", 4 * QROWS, QROWS),
              }
              if not POOL_CONV:
                  dve_feed[28] = ("c2", 5 * QROWS, QROWS)
                  dve_feed[36] = ("c2", 6 * QROWS, QROWS)
                  dve_feed[44] = ("c2", 7 * QROWS, QROWS)

              # ---- flash loop, software-pipelined emission ----
              # chunks 0 and 1 interleaved over tile-pairs (PE consumes each
              # fresh conv1 tile-pair twice, halving the demand rate while
              # DVE builds the rest), then chunks 2-7 sequentially.
              if int(os.environ.get("SEQ_GROUPS", "0")):
                  groups = [(qc, tt) for qc in range(NCHUNK) for tt in range(0, KT, 2)]
              else:
                  groups = [(qc, tt) for tt in range(0, KT, 2) for qc in (0, 1)]
                  groups += [(qc, tt) for qc in range(2, NCHUNK) for tt in range(0, KT, 2)]
              NGRP = len(groups)   # 64
              s2_of = {}
              out_ps_of = {}

              def emit_qk(gi):
                  qc, tt = groups[gi]
                  s2 = ps_s.tile([128, 2 * NQ], F32, tag="s2", name="s2")
                  s2_of[gi] = s2
                  for h in range(2):
                      t = tt + h
                      pi, off = tile_loc[t]
                      nc.tensor.matmul(
                          s2[:, h * NQ : (h + 1) * NQ],
                          lhsT=fr1p[pi][:, off : off + 128],
                          rhs=ft2c[qc][:, :],
                          start=True,
                          stop=True,
                      )

              l1_all = ps_l.tile([128, NQ], F32, tag="l1_all", name="l1_all")
              emit_qk(0)
              for gi in range(NGRP):
                  qc, tt = groups[gi]
                  if tt == 0:
                      out_ps_of[qc] = ps_o.tile([C, NQ], F32, tag="out_ps", name="out_ps")
                  out_ps = out_ps_of[qc]
                  lrow = 32 * (qc % 4)

                  # prefetch-emit the next group's QK so the PE FIFO never
                  # head-of-line blocks on this group's exp.  Ahead of it,
                  # drain one piece of the DVE conv feed.
                  if gi + 1 < NGRP:
                      if gi in dve_feed:
                          kind, j0, nrows = dve_feed[gi]
                          if kind == "c1":
                              conv1_piece(j0, nc.vector)
                          else:
                              conv2_rows(j0, nrows, nc.vector)
                      emit_qk(gi + 1)

                  s2 = s2_of.pop(gi)
                  p2 = pp.tile([128, 2 * NQ], BF16, tag="p2", name="p2")
                  nc.scalar.activation(p2[:, :], s2[:, :], AF.Exp)
                  for h in range(2):
                      t = tt + h
                      nc.tensor.matmul(
                          out_ps[:, :],
                          lhsT=vm[:, t * C : (t + 1) * C],
                          rhs=p2[:, h * NQ : (h + 1) * NQ],
                          start=(t == 0),
                          stop=(t == KT - 1),
                      )
                  # denominator: accumulate every tile's column-sums into one
                  # PSUM row (M=1 matmul costs the same as any other per
                  # column).  All 8 chunks share one PSUM bank, rotating over
                  # partitions {0,32,64,96} (chunks c and c+4 reuse a row ~40us
                  # apart, after the earlier chunk's staging copy).
                  for h in range(2):
                      t = tt + h
                      nc.tensor.matmul(
                          l1_all[lrow : lrow + 1, :],
                          lhsT=ones_col[:, :],
                          rhs=p2[:, h * NQ : (h + 1) * NQ],
                          start=(t == 0),
                          stop=(t == KT - 1),
                          tile_position=(0, lrow),
                      )

                  if tt == KT - 2:
                      # end of chunk: stage partials (bf16) for the collective;
                      # PSUM evacuation on DVE (GPSIMD cannot read PSUM on
                      # real hardware; DVE has slack once the conv feed winds
                      # down, and ps_o/ps_l double-buffering covers the lag)
                      o_sb = stg.tile([C, NQ], BF16, tag="o_sb", name="o_sb")
                      if int(os.environ.get("ACT_COPY", "0")):
                          nc.scalar.activation(o_sb[:, :], out_ps[:, :], AF.Copy)
                      else:
                          nc.vector.tensor_copy(o_sb[:, :], out_ps[:, :])
                      l1_sb = stg.tile([128, NQ], BF16, tag="l1_sb", name="l1_sb")
                      nc.vector.tensor_copy(
                          l1_sb[lrow : lrow + 1, :],
                          l1_all[lrow : lrow + 1, :],
                      )
                      base = qc * SROWS
                      nc.sync.dma_start(
                          out=stage_all[base : base + C, :], in_=o_sb[:, :]
                      )
                      nc.sync.dma_start(
                          out=stage_all[base + C : base + C + 1, :],
                          in_=l1_sb[lrow : lrow + 1, :],
                      )

          # ---- combine partials across cores; chunk i lands on core i ----
          red = dram.tile([SROWS, NQ], BF16)
          nc.gpsimd.collective_compute(
              "ReduceScatter",
              ALU.add,
              replica_groups=[list(range(NCORES))],
              ins=[stage_all[:, :]],
              outs=[red[:, :]],
          )

          # ---- normalize my chunk ----
          # numerator and (partition-broadcast) denominator row loaded
          # concurrently on the two HWDGE queues; the broadcast happens in
          # the DMA itself so no PE/matmul step is needed.
          osb = big.tile([C, NQ], BF16)
          nc.sync.dma_start(out=osb[:, :], in_=red[0:C, :])
          l1b = big.tile([C, NQ], BF16)
          nc.scalar.dma_start(
              out=l1b[:, :],
              in_=red[C : C + 1, :].partition_broadcast(128),
          )
          linv = big.tile([C, NQ], F32)
          nc.vector.reciprocal(linv[:, :], l1b[:, :])
          outf = big.tile([C, NQ], F32)
          nc.vector.tensor_mul(outf[:, :], osb[:, :], linv[:, :])
          nc.sync.dma_start(out=out_d[:, :], in_=outf[:, :])

    nc.finalize()
    return nc


def prep_inputs(feats_t, feats_ref, v_t, v_ref, conv1_w, conv1_b, conv2_w,
                conv2_b):
    bf = ml_dtypes.bfloat16
    ft = np.asarray(feats_t, np.float32)[0]            # (128, 64, 64)
    fr = np.asarray(feats_ref, np.float32)[0]          # (128, 4, 64, 64)
    vt = np.asarray(v_t, np.float32)[0, 0][::4, ::4]   # (64, 64)
    vr = np.asarray(v_ref, np.float32)[0, 0][:, ::4, ::4]  # (4, 64, 64)
    w1 = np.asarray(conv1_w, np.float32).reshape(9)
    w2 = np.asarray(conv2_w, np.float32).reshape(9)

    ftm_full = (ft * vt).astype(bf)                    # (128, 64, 64) masked
    frm_full = (fr * vr[None]).astype(bf)              # (128, 4, 64, 64) masked

    # full padded masked ft (shared by all cores)
    ftp = np.zeros((C, PW, PW), bf)
    ftp[:, 1:65, 1:65] = ftm_full
    ftp = ftp.reshape(C, FTPAD)

    in_maps = []
    for i in range(NCORES):
        r = i // 2
        y0 = (i % 2) * KROWS
        # padded local masked fr window: rows y0-1 .. y0+KROWS, 66 wide
        frp = np.zeros((C, KROWS + 2, PW), bf)
        ylo = max(0, y0 - 1)
        yhi = min(H, y0 + KROWS + 1)
        frp[:, (ylo - (y0 - 1)) : (yhi - (y0 - 1)), 1:65] = frm_full[:, r, ylo:yhi, :]

        # local masked V in (k%128, t, c) layout
        frl = frm_full[:, r, y0 : y0 + KROWS, :].reshape(C, NKL).astype(np.float32)
        vdev = np.ascontiguousarray(
            frl.reshape(C, KT, 128).transpose(2, 1, 0)
        ).reshape(128, NKL).astype(bf)

        in_maps.append({
            "frm": frp.reshape(C, KPAD),
            "ftm": ftp,
            "vm": vdev,
            "w1": w1,
            "w2": w2,
        })
    return in_maps


_CACHE = {}


def _get_runner():
    """Build the SPMD executable once; repeat kernel() calls reuse it."""
    if "fn" in _CACHE:
        return _CACHE["fn"]
    import jax
    from jax.sharding import Mesh, PartitionSpec
    from jax.experimental.shard_map import shard_map
    from concourse.bass2jax import (
        install_neuronx_cc_hook, _bass_exec_p, partition_id_tensor,
    )

    install_neuronx_cc_hook()
    nc = build_nc()
    pname = nc.partition_id_tensor.name if nc.partition_id_tensor else None
    in_names, out_names, out_avals, zero_outs = [], [], [], []
    for alloc in nc.m.functions[0].allocations:
        if not isinstance(alloc, mybir.MemoryLocationSet):
            continue
        name = alloc.memorylocations[0].name
        if alloc.kind == "ExternalInput":
            if name != pname:
                in_names.append(name)
        elif alloc.kind == "ExternalOutput":
            out_names.append(name)
            shape = tuple(alloc.tensor_shape)
            dtype = mybir.dt.np(alloc.dtype)
            out_avals.append(jax.core.ShapedArray(shape, dtype))
            zero_outs.append(np.zeros(shape, dtype))
    n_params = len(in_names)
    all_names = in_names + out_names + ([pname] if pname else [])

    def _body(*args):
        operands = list(args)
        if pname is not None:
            operands.append(partition_id_tensor())
        return tuple(_bass_exec_p.bind(
            *operands,
            out_avals=tuple(out_avals),
            in_names=tuple(all_names),
            out_names=tuple(out_names),
            lowering_input_output_aliases=(),
            sim_require_finite=True,
            sim_require_nnan=True,
            nc=nc,
        ))

    devices = jax.devices()[:NCORES]
    mesh = Mesh(np.asarray(devices), ("core",))
    n_outs = len(out_avals)
    fn = jax.jit(
        shard_map(
            _body, mesh=mesh,
            in_specs=(PartitionSpec("core"),) * (n_params + n_outs),
            out_specs=(PartitionSpec("core"),) * n_outs,
            check_rep=False,
        ),
        donate_argnums=tuple(range(n_params, n_params + n_outs)),
        keep_unused=True,
    )

    def run(in_maps):
        concat = [
            np.concatenate([np.asarray(m[n]) for m in in_maps], axis=0)
            for n in in_names
        ]
        zeros = [
            np.zeros((NCORES * z.shape[0], *z.shape[1:]), z.dtype)
            for z in zero_outs
        ]
        arrs = fn(*concat, *zeros)
        return [
            {
                name: np.asarray(arrs[i]).reshape(
                    NCORES, *out_avals[i].shape
                )[c]
                for i, name in enumerate(out_names)
            }
            for c in range(NCORES)
        ]

    _CACHE["fn"] = run
    return run


def kernel(**inputs) -> np.ndarray:
    run = _get_runner()
    in_maps = prep_inputs(**inputs)
    results = run(in_maps)
    out = np.empty((C, H * W), np.float32)
    for i in range(NCORES):
        out[:, i * NQ : (i + 1) * NQ] = results[i]["out"]
    return out.reshape(1, C, H, W)


# revision 36
# speedup vs baseline: 3.0133x; 1.4060x over previous
"""Trainium2 Bass kernel for nn_CorrelationMatrix (sparse_attention).

Math: the reference builds a (b, r, h_t*w_t, h_r*w_r) correlation volume,
runs a pair of 3x3 convs over it (first over the (h_r, w_r) key grid, then
over the (h_t, w_t) query grid), a joint softmax over (r, h_r, w_r) per
query, and aggregates masked reference features.

Because the convs are linear and each acts on one side of the einsum, they
commute into the feature tensors:

    conv1 over keys    -> applied to K features:  K = conv1(fr * vr)
    conv2 over queries -> applied to Q features:  Q = conv2(ft * vt)

and the conv biases only add per-query constants, which cancel exactly in
the softmax.  The whole module collapses to flash attention:

    S = Q^T K          (4096 queries x 16384 keys, d=128)
    P = exp(S)         (no max-subtraction: |S| < ~3 by construction)
    out = V P / sum_k P,   V = fr*vr

Sharding: KEYS are sharded 8 ways (core i gets ref frame i//2, row-half
i%2 = 2048 keys); every core runs all 4096 queries against its local keys,
accumulating partial sum_k exp()*V and partial denominators.  One
ReduceScatter(add) combines the partials and lands chunk i of the queries
on core i, which normalizes and emits out[:, 512*i : 512*(i+1)].

Schedule notes (v4, hardware-validated):
 - the mask multiplies (fr*vr, ft*vt, V masking) are folded into host-side
   prep: the device receives pre-masked frm/ftm/vm, removing ~5us of DVE
   work and two large broadcast DMAs per iteration.
 - ALL conv runs on DVE as tensor_scalar_mul + tensor_add pairs (4x/2x DVE
   perf modes).  Hardware A/B showed the alternatives are mis-modeled and
   far slower in reality: gpsimd TensorTensor convs cost ~180us/iter extra,
   a fused scalar_tensor_tensor tap ~50us extra, and Act-engine staging
   copies ~90us extra.  Pool and (mostly) Act stay out of the data path.
 - the first two query chunks are interleaved over key tile-pairs
   ((0,t),(1,t),...) so the PE consumes fresh conv1 tiles at half rate
   during the warmup while DVE builds them; chunks 2-7 then run
   sequentially (hardware A/B: the interleave is worth ~tens of us/iter).
   The DVE conv stream is emitted as a "feed" at specific group indices so
   per-engine FIFO order tracks consumption order.
 - exact dependencies: inputs are DMA'd as need-ordered pieces into
   separate SBUF tiles (2-row overlap re-read), and conv outputs get one
   tile per conv1 piece / per query chunk, so no consumer waits on an
   unrelated producer.
 - denominators: per chunk, 16 M=1 ones-matmuls accumulate into one PSUM
   row (PE cost is per-column regardless of M; the old tile_position
   4-batch gives no concurrency).  All 8 chunks share a single PSUM bank,
   rotating partitions {0,32,64,96}, freeing a bank for ps_o bufs=3 so
   staging copies are never deadline-critical.
 - per-iteration tiles live in a bufs=2 pool so iteration i+1's DMAs and
   conv prologue overlap iteration i's flash tail inside the timing loop.
 - flash loop is software-pipelined at emission level: the next group's QK
   matmuls are emitted before this group's PV so the PE FIFO never
   head-of-line blocks on the Act exp; exp covers two key tiles (two PSUM
   banks, 1024 wide) per instruction.
"""

import os
import numpy as np
import ml_dtypes

import concourse.bass as bass
import concourse.tile as tile
from concourse import bacc, mybir
from concourse.bass_utils import run_bass_kernel_spmd

BF16 = mybir.dt.bfloat16
F32 = mybir.dt.float32
AF = mybir.ActivationFunctionType
ALU = mybir.AluOpType

C = 128          # channels (= contraction dim = SBUF partitions)
R = 4            # reference frames
H = W = 64       # spatial grid
HW = H * W       # 4096
NK = R * HW      # 16384 keys total
NCORES = 8
NQ = 512              # queries per output chunk (and per core's RS slice)
PW = 66               # padded width for 3x3 conv (1 zero col each side)
KROWS = 32            # key rows per core
KPAD = (KROWS + 2) * PW   # 2244: padded local fr window (1 halo row each side)
NKL = KROWS * W       # 2048 local keys
KT = NKL // 128       # 16 local key tiles
QROWS = 8             # query rows per chunk
FTPAD = PW * PW       # 4356: full padded ft
NCHUNK = 8            # query chunks (one per core in the RS)
SROWS = C + 1         # stage rows per chunk: 128 out + 1 denominator row


def build_nc(loop_n: int = 1):
    nc = bacc.Bacc(None, target_bir_lowering=False, debug=False)

    frm_d = nc.declare_dram_parameter("frm", [C, KPAD], BF16, isOutput=False)
    ftm_d = nc.declare_dram_parameter("ftm", [C, FTPAD], BF16, isOutput=False)
    vm_d = nc.declare_dram_parameter("vm", [128, NKL], BF16, isOutput=False)
    w1_d = nc.declare_dram_parameter("w1", [9], F32, isOutput=False)
    w2_d = nc.declare_dram_parameter("w2", [9], F32, isOutput=False)
    out_d = nc.declare_dram_parameter("out", [C, NQ], F32, isOutput=True)

    with tile.TileContext(nc) as tc:
        with (
            tc.tile_pool(name="big", bufs=1) as big,
            tc.tile_pool(name="db", bufs=2) as db,
            tc.tile_pool(name="pp", bufs=4) as pp,
            tc.tile_pool(name="stg", bufs=3) as stg,
            tc.tile_pool(name="ps_s", bufs=2, space="PSUM") as ps_s,
            tc.tile_pool(name="ps_o", bufs=3, space="PSUM") as ps_o,
            tc.tile_pool(name="ps_l", bufs=1, space="PSUM") as ps_l,
            tc.tile_pool(name="dram", bufs=1, space="DRAM") as dram,
        ):
          import contextlib
          # constants, allocated once outside the timing loop
          ones_col = big.tile([128, 1], BF16)
          nc.vector.memset(ones_col[:, :], 1.0)
          stage_all = dram.tile([NCHUNK * SROWS, NQ], BF16)
          loop_cm = tc.For_i(0, loop_n, 1) if loop_n > 1 else contextlib.nullcontext()
          with loop_cm:
              # input loads: HWDGE descriptor-gen and the DMA engine are both
              # effectively serial shared resources, AND dependency tracking
              # for DMA-written tiles is whole-tile, so each need-ordered
              # piece gets its OWN SBUF tile (with a 2-row overlap re-read so
              # every conv piece reads exactly one input tile).
              w1_sb = db.tile([128, 9], F32)
              w2_sb = db.tile([128, 9], F32)
              FRA = 14     # frm rows 0-13 -> frma; rows 12-33 -> frmb
              FTA = 10     # ftm rows 0-9 -> ftma; rows 8-65 -> ftmb
              frma = db.tile([C, FRA * PW], BF16)
              frmb = db.tile([C, (KROWS + 2 - FRA + 2) * PW], BF16)
              ftma = db.tile([C, FTA * PW], BF16)
              ftmb = db.tile([C, (PW - FTA + 2) * PW], BF16)
              vm = db.tile([128, NKL], BF16)
              nc.sync.dma_start(
                  out=frma[:, :], in_=frm_d[:, 0 : FRA * PW])
              nc.scalar.dma_start(
                  out=w1_sb[:, :],
                  in_=bass.AP(tensor=w1_d, offset=0, ap=[[0, 128], [1, 9]]),
              )
              nc.scalar.dma_start(
                  out=w2_sb[:, :],
                  in_=bass.AP(tensor=w2_d, offset=0, ap=[[0, 128], [1, 9]]),
              )
              nc.scalar.dma_start(
                  out=ftma[:, :], in_=ftm_d[:, 0 : FTA * PW])
              nc.sync.dma_start(
                  out=frmb[:, :], in_=frm_d[:, (FRA - 2) * PW : KPAD])
              nc.scalar.dma_start(
                  out=ftmb[:, :], in_=ftm_d[:, (FTA - 2) * PW : FTPAD])
              nc.scalar.dma_start(out=vm[:, :], in_=vm_d[:, :])

              frma3 = frma[:, :].rearrange("p (r c) -> p r c", c=PW)
              frmb3 = frmb[:, :].rearrange("p (r c) -> p r c", c=PW)
              ftma3 = ftma[:, :].rearrange("p (r c) -> p r c", c=PW)
              ftmb3 = ftmb[:, :].rearrange("p (r c) -> p r c", c=PW)

              # conv outputs also get one tile per piece / per query chunk so
              # every consumer's dependency is exact.
              C1_PIECES = [(0, 4), (4, 8), (12, 8), (20, 8), (28, 4)]
              fr1p = [
                  db.tile([C, n * W], BF16, name=f"fr1p{i}")
                  for i, (_, n) in enumerate(C1_PIECES)
              ]
              fr1pv = [
                  t[:, :].rearrange("p (j x) -> p j x", x=W) for t in fr1p
              ]
              # key tile t (rows 2t, 2t+1) -> (piece index, column offset)
              tile_loc = {}
              for pi, (j0, n) in enumerate(C1_PIECES):
                  for t in range(j0 // 2, (j0 + n) // 2):
                      tile_loc[t] = (pi, (2 * t - j0) * W)
              ft2c = [
                  db.tile([C, NQ], BF16, name=f"ft2c{i}")
                  for i in range(NCHUNK)
              ]
              ft2cv = [
                  t[:, :].rearrange("p (j x) -> p j x", x=W) for t in ft2c
              ]
              # ONE shared tmp for both convs on DVE: the WAR chain through it
              # pins the DVE conv stream to emission order (the Tile scheduler
              # otherwise interleaves the independent streams, delaying the
              # completion of every piece)
              tmp_k = db.tile([C, 8 * W], BF16)
              tmp_kv = tmp_k[:, :].rearrange("p (j x) -> p j x", x=W)
              tmp_p = db.tile([C, 8 * W], BF16)
              tmp_pv = tmp_p[:, :].rearrange("p (j x) -> p j x", x=W)

              def conv_piece(dstv, d0, src3, s0, w_sb, j0, nrows, eng):
                  # conv output rows [j0, j0+nrows) into dstv rows j0-d0...;
                  # src3 holds input rows starting at absolute row s0.
                  # On DVE use mul(4x mode) + add(2x mode) pairs, on Pool the
                  # fused form (flat cost there).
                  dst = dstv[:, j0 - d0 : j0 - d0 + nrows, :]
                  on_pool = eng is nc.gpsimd
                  taps = [1, 2, 3, 4, 5, 6, 7, 8, 0]
                  if int(os.environ.get("TAP0_FIRST", "1")):
                      taps = [0, 1, 2, 3, 4, 5, 6, 7, 8]
                  for ti, tap in enumerate(taps):
                      dy, dx = divmod(tap, 3)
                      src = src3[
                          :, j0 + dy - s0 : j0 + dy - s0 + nrows, dx : dx + W
                      ]
                      wap = w_sb[:, tap : tap + 1]
                      if on_pool:
                          # GPSIMD runs only TensorTensor/Memset on real hw:
                          # multiply by a free-broadcast view of the weight
                          wb = wap.rearrange("p (a b) -> p a b", a=1)
                          wb = wb.broadcast_to([128, nrows, W])
                          if ti == 0:
                              eng.tensor_mul(dst, src, wb)
                          else:
                              tv = tmp_pv[:, 0:nrows, :]
                              eng.tensor_mul(tv, src, wb)
                              eng.tensor_add(dst, dst, tv)
                      elif ti == 0 and tap != 0:
                          eng.tensor_scalar_mul(dst, src, wap)
                      elif ti == 0 and tap == 0:
                          eng.tensor_scalar_mul(dst, src, wap)
                      elif tap == 0:
                          # last tap fused (dst += src*w): slower per-op but
                          # keeps every DVE op on the piece's tmp/dst chain so
                          # the scheduler cannot hoist it ahead of its inputs
                          eng.scalar_tensor_tensor(
                              dst, src, wap, dst, ALU.mult, ALU.add
                          )
                      else:
                          tv = tmp_kv[:, 0:nrows, :]
                          eng.tensor_scalar_mul(tv, src, wap)
                          eng.tensor_add(dst, dst, tv)

              def conv1_piece(pi, eng):
                  j0, n = C1_PIECES[pi]
                  src3, s0 = (frma3, 0) if j0 + n + 1 < FRA else (frmb3, FRA - 2)
                  conv_piece(fr1pv[pi], j0, src3, s0, w1_sb, j0, n, eng)

              def conv2_rows(j0, nrows, eng):
                  qc = j0 // QROWS
                  src3, s0 = (ftma3, 0) if j0 + nrows + 1 < FTA else (ftmb3, FTA - 2)
                  conv_piece(
                      ft2cv[qc], qc * QROWS, src3, s0, w2_sb, j0, nrows, eng
                  )

              # ---- conv prologue ----
              # Pool stream (independent FIFO): the three LAST query chunks -
              # TensorTensor-based taps are ~4x slower than DVE's, but these
              # aren't consumed until ~60-80us in, and Pool runs concurrently.
              POOL_CONV = int(os.environ.get("POOL_CONV", "0"))
              if POOL_CONV:
                  for pc in (5, 6, 7):
                      conv2_rows(pc * QROWS, QROWS, nc.gpsimd)
              # DVE prologue: tiles 0,1 -> chunk 0.  The rest of the DVE conv
              # stream is fed at group boundaries so emission order tracks
              # consumption order; chunks 3/4 are deferred until after the
              # first staging copies so the ps_o/ps_l buffers recycle in time.
              conv1_piece(0, nc.vector)
              conv2_rows(0, QROWS, nc.vector)

              dve_feed = {
                  0: ("c2", 1 * QROWS, QROWS),
                  1: ("c1", 1, 0),
                  2: ("c1", 2, 0),
                  3: ("c1", 3, 0),
                  4: ("c1", 4, 0),
                  5: ("c2", 2 * QROWS, QROWS),
                  12: ("c2", 3 * QROWS, QROWS),
                  20: ("c2", 4 * QROWS, QROWS),
              }
              if not POOL_CONV:
                  dve_feed[28] = ("c2", 5 * QROWS, QROWS)
                  dve_feed[36] = ("c2", 6 * QROWS, QROWS)
                  dve_feed[44] = ("c2", 7 * QROWS, QROWS)

              # ---- flash loop, software-pipelined emission ----
              # chunks 0 and 1 interleaved over tile-pairs (PE consumes each
              # fresh conv1 tile-pair twice, halving the demand rate while
              # DVE builds the rest), then chunks 2-7 sequentially.
              if int(os.environ.get("SEQ_GROUPS", "0")):
                  groups = [(qc, tt) for qc in range(NCHUNK) for tt in range(0, KT, 2)]
              else:
                  groups = [(qc, tt) for tt in range(0, KT, 2) for qc in (0, 1)]
                  groups += [(qc, tt) for qc in range(2, NCHUNK) for tt in range(0, KT, 2)]
              NGRP = len(groups)   # 64
              s2_of = {}
              out_ps_of = {}

              def emit_qk(gi):
                  qc, tt = groups[gi]
                  s2 = ps_s.tile([128, 2 * NQ], F32, tag="s2", name="s2")
                  s2_of[gi] = s2
                  for h in range(2):
                      t = tt + h
                      pi, off = tile_loc[t]
                      nc.tensor.matmul(
                          s2[:, h * NQ : (h + 1) * NQ],
                          lhsT=fr1p[pi][:, off : off + 128],
                          rhs=ft2c[qc][:, :],
                          start=True,
                          stop=True,
                      )

              l1_all = ps_l.tile([128, NQ], F32, tag="l1_all", name="l1_all")
              emit_qk(0)
              for gi in range(NGRP):
                  qc, tt = groups[gi]
                  if tt == 0:
                      out_ps_of[qc] = ps_o.tile([C, NQ], F32, tag="out_ps", name="out_ps")
                  out_ps = out_ps_of[qc]
                  lrow = 32 * (qc % 4)

                  # prefetch-emit the next group's QK so the PE FIFO never
                  # head-of-line blocks on this group's exp.  Ahead of it,
                  # drain one piece of the DVE conv feed.
                  if gi + 1 < NGRP:
                      if gi in dve_feed:
                          kind, j0, nrows = dve_feed[gi]
                          if kind == "c1":
                              conv1_piece(j0, nc.vector)
                          else:
                              conv2_rows(j0, nrows, nc.vector)
                      emit_qk(gi + 1)

                  s2 = s2_of.pop(gi)
                  p2 = pp.tile([128, 2 * NQ], BF16, tag="p2", name="p2")
                  nc.scalar.activation(p2[:, :], s2[:, :], AF.Exp)
                  for h in range(2):
                      t = tt + h
                      nc.tensor.matmul(
                          out_ps[:, :],
                          lhsT=vm[:, t * C : (t + 1) * C],
                          rhs=p2[:, h * NQ : (h + 1) * NQ],
                          start=(t == 0),
                          stop=(t == KT - 1),
                      )
                  # denominator: accumulate every tile's column-sums into one
                  # PSUM row (M=1 matmul costs the same as any other per
                  # column).  All 8 chunks share one PSUM bank, rotating over
                  # partitions {0,32,64,96} (chunks c and c+4 reuse a row ~40us
                  # apart, after the earlier chunk's staging copy).
                  for h in range(2):
                      t = tt + h
                      nc.tensor.matmul(
                          l1_all[lrow : lrow + 1, :],
                          lhsT=ones_col[:, :],
                          rhs=p2[:, h * NQ : (h + 1) * NQ],
                          start=(t == 0),
                          stop=(t == KT - 1),
                          tile_position=(0, lrow),
                      )

                  if tt == KT - 2:
                      # end of chunk: stage partials (bf16) for the collective;
                      # PSUM evacuation on DVE (GPSIMD cannot read PSUM on
                      # real hardware; DVE has slack once the conv feed winds
                      # down, and ps_o/ps_l double-buffering covers the lag)
                      o_sb = stg.tile([C, NQ], BF16, tag="o_sb", name="o_sb")
                      if int(os.environ.get("ACT_COPY", "0")):
                          nc.scalar.activation(o_sb[:, :], out_ps[:, :], AF.Copy)
                      else:
                          nc.vector.tensor_copy(o_sb[:, :], out_ps[:, :])
                      l1_sb = stg.tile([128, NQ], BF16, tag="l1_sb", name="l1_sb")
                      nc.vector.tensor_copy(
                          l1_sb[lrow : lrow + 1, :],
                          l1_all[lrow : lrow + 1, :],
                      )
                      base = qc * SROWS
                      nc.sync.dma_start(
                          out=stage_all[base : base + C, :], in_=o_sb[:, :]
                      )
                      nc.sync.dma_start(
                          out=stage_all[base + C : base + C + 1, :],
                          in_=l1_sb[lrow : lrow + 1, :],
                      )

          # ---- combine partials across cores; chunk i lands on core i ----
          red = dram.tile([SROWS, NQ], BF16)
          nc.gpsimd.collective_compute(
              "ReduceScatter",
              ALU.add,
              replica_groups=[list(range(NCORES))],
              ins=[stage_all[:, :]],
              outs=[red[:, :]],
          )

          # ---- normalize my chunk ----
          # numerator and (partition-broadcast) denominator row loaded
          # concurrently on the two HWDGE queues; the broadcast happens in
          # the DMA itself so no PE/matmul step is needed.
          osb = big.tile([C, NQ], BF16)
          nc.sync.dma_start(out=osb[:, :], in_=red[0:C, :])
          l1b = big.tile([C, NQ], BF16)
          nc.scalar.dma_start(
              out=l1b[:, :],
              in_=red[C : C + 1, :].partition_broadcast(128),
          )
          linv = big.tile([C, NQ], F32)
          nc.vector.reciprocal(linv[:, :], l1b[:, :])
          outf = big.tile([C, NQ], F32)
          nc.vector.tensor_mul(outf[:, :], osb[:, :], linv[:, :])
          nc.sync.dma_start(out=out_d[:, :], in_=outf[:, :])

    nc.finalize()
    return nc


def prep_inputs(feats_t, feats_ref, v_t, v_ref, conv1_w, conv1_b, conv2_w,
                conv2_b):
    bf = ml_dtypes.bfloat16
    ft = np.asarray(feats_t, np.float32)[0]            # (128, 64, 64)
    fr = np.asarray(feats_ref, np.float32)[0]          # (128, 4, 64, 64)
    vt = np.asarray(v_t, np.float32)[0, 0][::4, ::4]   # (64, 64)
    vr = np.asarray(v_ref, np.float32)[0, 0][:, ::4, ::4]  # (4, 64, 64)
    w1 = np.asarray(conv1_w, np.float32).reshape(9)
    w2 = np.asarray(conv2_w, np.float32).reshape(9)

    ftm_full = (ft * vt).astype(bf)                    # (128, 64, 64) masked
    frm_full = (fr * vr[None]).astype(bf)              # (128, 4, 64, 64) masked

    # full padded masked ft (shared by all cores)
    ftp = np.zeros((C, PW, PW), bf)
    ftp[:, 1:65, 1:65] = ftm_full
    ftp = ftp.reshape(C, FTPAD)

    in_maps = []
    for i in range(NCORES):
        r = i // 2
        y0 = (i % 2) * KROWS
        # padded local masked fr window: rows y0-1 .. y0+KROWS, 66 wide
        frp = np.zeros((C, KROWS + 2, PW), bf)
        ylo = max(0, y0 - 1)
        yhi = min(H, y0 + KROWS + 1)
        frp[:, (ylo - (y0 - 1)) : (yhi - (y0 - 1)), 1:65] = frm_full[:, r, ylo:yhi, :]

        # local masked V in (k%128, t, c) layout
        frl = frm_full[:, r, y0 : y0 + KROWS, :].reshape(C, NKL).astype(np.float32)
        vdev = np.ascontiguousarray(
            frl.reshape(C, KT, 128).transpose(2, 1, 0)
        ).reshape(128, NKL).astype(bf)

        in_maps.append({
            "frm": frp.reshape(C, KPAD),
            "ftm": ftp,
            "vm": vdev,
            "w1": w1,
            "w2": w2,
        })
    return in_maps


_CACHE = {}


def _get_runner():
    """Build the SPMD executable once; repeat kernel() calls reuse it."""
    if "fn" in _CACHE:
        return _CACHE["fn"]
    import jax
    from jax.sharding import Mesh, PartitionSpec
    from jax.experimental.shard_map import shard_map
    from concourse.bass2jax import (
        install_neuronx_cc_hook, _bass_exec_p, partition_id_tensor,
    )

    install_neuronx_cc_hook()
    nc = build_nc()
    pname = nc.partition_id_tensor.name if nc.partition_id_tensor else None
    in_names, out_names, out_avals, zero_outs = [], [], [], []
    for alloc in nc.m.functions[0].allocations:
        if not isinstance(alloc, mybir.MemoryLocationSet):
            continue
        name = alloc.memorylocations[0].name
        if alloc.kind == "ExternalInput":
            if name != pname:
                in_names.append(name)
        elif alloc.kind == "ExternalOutput":
            out_names.append(name)
            shape = tuple(alloc.tensor_shape)
            dtype = mybir.dt.np(alloc.dtype)
            out_avals.append(jax.core.ShapedArray(shape, dtype))
            zero_outs.append(np.zeros(shape, dtype))
    n_params = len(in_names)
    all_names = in_names + out_names + ([pname] if pname else [])

    def _body(*args):
        operands = list(args)
        if pname is not None:
            operands.append(partition_id_tensor())
        return tuple(_bass_exec_p.bind(
            *operands,
            out_avals=tuple(out_avals),
            in_names=tuple(all_names),
            out_names=tuple(out_names),
            lowering_input_output_aliases=(),
            sim_require_finite=True,
            sim_require_nnan=True,
            nc=nc,
        ))

    devices = jax.devices()[:NCORES]
    mesh = Mesh(np.asarray(devices), ("core",))
    n_outs = len(out_avals)
    fn = jax.jit(
        shard_map(
            _body, mesh=mesh,
            in_specs=(PartitionSpec("core"),) * (n_params + n_outs),
            out_specs=(PartitionSpec("core"),) * n_outs,
            check_rep=False,
        ),
        donate_argnums=tuple(range(n_params, n_params + n_outs)),
        keep_unused=True,
    )

    def run(in_maps):
        concat = [
            np.concatenate([np.asarray(m[n]) for m in in_maps], axis=0)
            for n in in_names
        ]
        zeros = [
            np.zeros((NCORES * z.shape[0], *z.shape[1:]), z.dtype)
            for z in zero_outs
        ]
        arrs = fn(*concat, *zeros)
        return [
            {
                name: np.asarray(arrs[i]).reshape(
                    NCORES, *out_avals[i].shape
                )[c]
                for i, name in enumerate(out_names)
            }
            for c in range(NCORES)
        ]

    _CACHE["fn"] = run
    return run


def kernel(**inputs) -> np.ndarray:
    run = _get_runner()
    in_maps = prep_inputs(**inputs)
    results = run(in_maps)
    out = np.empty((C, H * W), np.float32)
    for i in range(NCORES):
        out[:, i * NQ : (i + 1) * NQ] = results[i]["out"]
    return out.reshape(1, C, H, W)
